# revision 1
# baseline (speedup 1.0000x reference)
"""Int-infer matmul kernel for trn2, 8 NeuronCores, data-parallel over (b,h).

reference: y = clip(round(matmul(clip(round(x1*r1)), clip(round(x2*r2))) / 16), -128, 127)
shapes: x1 [2,16,2048,64] f32, x2 [2,16,64,2048] f32 -> y [2,16,2048,2048] f32

Strategy (per core, 4 of the 32 (b,h) pairs):
 - rescale: f32 -> *r -> int8 (HW convert = RNE + saturate == clip(round(.)))
 - int8 -> bf16 (exact for [-128,127]); bf16 matmul accumulates exactly in f32 PSUM
 - x1 transposed on PE (col-tiled so pair A -> psum partitions 0:64, B -> 64:128)
 - main matmuls row-packed: two K=64 matmuls (pairs A,B) run concurrently via
   tile_position (0,0)/(64,0)
 - evict psum f32 -> *1/16 -> int8 (RNE+sat == clip(round(y/16))), alternating
   DVE/ACT; int8 output DMA'd out (4x fewer bytes), upcast to f32 on host
"""
import sys

sys.path.insert(0, "/opt/trn_rl_repo")

import numpy as np
import concourse.bass as bass
import concourse.bacc as bacc
import concourse.mybir as mybir
import concourse.tile as tile
from concourse.bass_utils import run_bass_kernel_spmd
from concourse.masks import make_identity

F32 = mybir.dt.float32
BF16 = mybir.dt.bfloat16
I8 = mybir.dt.int8
AF = mybir.ActivationFunctionType

N_CORES = 8
PAIRS_PER_CORE = 4  # 2*16 = 32 (b,h) pairs / 8 cores
S = 2048
D = 64
N_MM = 512  # moving free dim per matmul
INV_G = 1.0 / 16.0


def build_program(r1: float, r2: float, repeat: int = 1) -> bass.Bass:
    nc = bacc.Bacc("TRN2", target_bir_lowering=False, debug=False, num_devices=N_CORES)
    x1 = nc.dram_tensor("x1", [PAIRS_PER_CORE, S, D], F32, kind="ExternalInput").ap()
    x2 = nc.dram_tensor("x2", [PAIRS_PER_CORE, D, S], F32, kind="ExternalInput").ap()
    y = nc.dram_tensor("y", [PAIRS_PER_CORE, S, S], I8, kind="ExternalOutput").ap()

    n_ss = PAIRS_PER_CORE // 2  # supersteps, 2 pairs each (A on partitions 0:64, B on 64:128)
    n_mchunk = S // 128  # 16 m-chunks of 128 rows
    if repeat > 1:
        # distinct input shape per repeat-count so jax's compilation cache
        # cannot collide programs that differ only in the BIR payload
        nc.dram_tensor("rep_marker", [1, repeat], F32, kind="ExternalInput")

    with tile.TileContext(nc) as tc:
      for _rep in range(repeat):
        with (
            tc.tile_pool(name="const", bufs=1) as const_pool,
            tc.tile_pool(name="x1raw", bufs=3) as x1raw_pool,
            tc.tile_pool(name="x1i8", bufs=2) as x1i8_pool,
            tc.tile_pool(name="x1bf", bufs=2) as x1bf_pool,
            tc.tile_pool(name="x2raw", bufs=3) as x2raw_pool,
            tc.tile_pool(name="x2i8", bufs=2) as x2i8_pool,
            tc.tile_pool(name="x2bf", bufs=2) as x2bf_pool,
            tc.tile_pool(name="x1T", bufs=2) as x1T_pool,
            tc.tile_pool(name="ostage", bufs=6) as ostage_pool,
            tc.tile_pool(name="tpsum", bufs=2, space="PSUM") as tpsum_pool,
            tc.tile_pool(name="mpsum", bufs=3, space="PSUM") as mpsum_pool,
        ):
            identity = const_pool.tile([128, 128], BF16)
            make_identity(nc, identity)
            ev = {"act": 0.0, "dve": 0.0}

            def input_loads(ss):
                pa, pb = 2 * ss, 2 * ss + 1
                x2r = x2raw_pool.tile([128, S], F32, tag="x2raw")
                nc.sync.dma_start(out=x2r[0:64, :], in_=x2[pa])
                nc.sync.dma_start(out=x2r[64:128, :], in_=x2[pb])
                x1rs = []
                h = n_mchunk // 2
                for p in (pa, pb):
                    x1r = x1raw_pool.tile([128, n_mchunk * D], F32, tag="x1raw")
                    dst = x1r.rearrange("p (c d) -> p c d", c=n_mchunk)
                    srcv = x1[p].rearrange("(c p) d -> p c d", p=128)
                    nc.sync.dma_start(out=dst[:, 0:h, :], in_=srcv[:, 0:h, :])
                    nc.sync.dma_start(out=dst[:, h:, :], in_=srcv[:, h:, :])
                    x1rs.append(x1r)
                return x2r, x1rs

            def assign(cost_act, cost_dve):
                # deficit-weighted ACT/DVE balancing (returns engine + books cost)
                if ev["act"] + cost_act <= ev["dve"] + cost_dve:
                    ev["act"] += cost_act
                    return "act"
                ev["dve"] += cost_dve
                return "dve"

            def prep_compute(ss, x2r, x1rs, use_pool):
                # rescale f32 -> *r -> int8 (RNE+sat), convert int8 -> bf16.
                # ss0's x2 chain is the ramp critical path (first MM waits on
                # x2b) - run it on the then-idle DVE/ACT; ss1 overlaps main0
                # so its x2 goes to GPSIMD, off the evict engines.
                x2i = x2i8_pool.tile([128, S], I8, tag="x2i8")
                x2b = x2bf_pool.tile([128, S], BF16, tag="x2bf")
                nc.vector.tensor_scalar_mul(x2i[:], x2r[:], r2)
                nc.scalar.activation(x2b[:], x2i[:], AF.Copy)
                ev["dve"] += 1133.0
                ev["act"] += 1949.0
                x1bfs = []
                for x1r in x1rs:
                    x1i = x1i8_pool.tile([128, n_mchunk * D], I8, tag="x1i8")
                    x1b = x1bf_pool.tile([128, n_mchunk * D], BF16, tag="x1bf")
                    # real GPSIMD is far slower than modeled (v7 lesson):
                    # all prep on DVE/ACT, deficit-booked
                    nc.vector.tensor_scalar_mul(x1i[:], x1r[:], r1)
                    nc.scalar.activation(x1b[:], x1i[:], AF.Copy)
                    ev["dve"] += 664.0
                    ev["act"] += 1095.0
                    x1bfs.append(x1b)
                # PE transpose x1 [128(s),64(d)] chunks -> x1T [64(d),128(s)];
                # pair A -> psum partitions 0:64 (cols 0:64), pair B -> 64:128.
                # One x1T tile per 4-chunk group so main matmuls for m-chunks
                # 4g..4g+3 depend only on group g's copy (earlier MM start).
                x1Ts = []
                for g in range(n_mchunk // 4):
                    tp = tpsum_pool.tile([128, 512], BF16, tag="tpsum")
                    for j in range(4):
                        c = g * 4 + j
                        nc.tensor.transpose(
                            tp[0:64, j * 128:(j + 1) * 128],
                            x1bfs[0][:, c * D:(c + 1) * D],
                            identity[:],
                            tile_position=(0, 0),
                        )
                        nc.tensor.transpose(
                            tp[64:128, j * 128:(j + 1) * 128],
                            x1bfs[1][:, c * D:(c + 1) * D],
                            identity[:],
                            tile_position=(0, 64),
                        )
                    x1T = x1T_pool.tile([128, 512], BF16, tag=f"x1T{g}")
                    if assign(669.0, 462.0) == "act":
                        nc.scalar.activation(x1T[:], tp[:], AF.Copy)
                    else:
                        nc.vector.tensor_copy(x1T[:], tp[:])
                    x1Ts.append(x1T)
                return x1Ts, x2b

            def main(ss, x1Ts, x2b):
                for mp in range(n_mchunk // 2):
                    for half, p in ((0, 2 * ss), (1, 2 * ss + 1)):
                        lo, hi = half * 64, half * 64 + 64
                        # one staging tile + one output DMA covers 2 m-chunks
                        ost = ostage_pool.tile([128, 2 * S], I8, tag="ostage")
                        for mm in range(2):
                            m = 2 * mp + mm
                            x1T = x1Ts[m // 4]
                            moff = (m % 4) * 128
                            for nn in range(S // 1024):
                                ps = mpsum_pool.tile([128, 1024], F32, tag="mpsum")
                                for k in range(2):
                                    n0 = nn * 1024 + k * N_MM
                                    nc.tensor.matmul(
                                        ps[:, k * N_MM:(k + 1) * N_MM],
                                        lhsT=x1T[lo:hi, moff:moff + 128],
                                        rhs=x2b[lo:hi, n0:n0 + N_MM],
                                        start=True,
                                        stop=True,
                                        tile_position=(half * 64, 0),
                                    )
                                dst = ost[:, mm * S + nn * 1024:mm * S + (nn + 1) * 1024]
                                # evict: *1/16 then f32->int8 (RNE+sat); deficit-
                                # weighted ACT/DVE split (ACT cheaper per elem)
                                if assign(1095.0, 1262.0) == "act":
                                    nc.scalar.activation(dst, ps[:], AF.Copy, scale=INV_G)
                                else:
                                    nc.vector.tensor_scalar_mul(dst, ps[:], INV_G)
                        nc.sync.dma_start(
                            out=y[p, 2 * mp * 128:(2 * mp + 2) * 128, :].rearrange(
                                "(r p) c -> p r c", p=128
                            ),
                            in_=ost.rearrange("p (r c) -> p r c", r=2),
                        )

            loads0 = input_loads(0)
            p0 = prep_compute(0, *loads0, use_pool=False)
            loads1 = input_loads(1)
            main(0, *p0)
            p1 = prep_compute(1, *loads1, use_pool=True)
            main(1, *p1)

    nc.compile()
    return nc


_CACHE: dict = {}


def kernel(x1, x2, scale1_last_layer, scale_x1, scale2_last_layer, scale_x2):
    x1 = np.asarray(x1, dtype=np.float32)
    x2 = np.asarray(x2, dtype=np.float32)
    # same fp32 division the reference performs
    r1 = float(np.float32(scale1_last_layer) / np.float32(scale_x1))
    r2 = float(np.float32(scale2_last_layer) / np.float32(scale_x2))

    key = (r1, r2)
    if key not in _CACHE:
        _CACHE[key] = build_program(r1, r2)
    nc = _CACHE[key]

    b, h = x1.shape[0], x1.shape[1]
    x1r = x1.reshape(b * h, S, D)
    x2r = x2.reshape(b * h, D, S)
    in_maps = [
        {
            "x1": np.ascontiguousarray(x1r[c * PAIRS_PER_CORE:(c + 1) * PAIRS_PER_CORE]),
            "x2": np.ascontiguousarray(x2r[c * PAIRS_PER_CORE:(c + 1) * PAIRS_PER_CORE]),
        }
        for c in range(N_CORES)
    ]
    res = run_bass_kernel_spmd(nc, in_maps, list(range(N_CORES)))
    out = np.concatenate([r["y"] for r in res.results], axis=0)
    return out.reshape(b, h, S, S).astype(np.float32)


if __name__ == "__main__":
    # smoke test with random data
    rng = np.random.default_rng(0)
    x1 = np.round(np.clip(rng.normal(size=(2, 16, S, D)) * 40.0, -128, 127)).astype(np.float32)
    x2 = np.round(np.clip(rng.normal(size=(2, 16, D, S)) * 40.0, -128, 127)).astype(np.float32)
    y = kernel(x1, x2, np.float32(0.1), np.float32(0.05), np.float32(0.08), np.float32(0.04))
    print("out", y.shape, y.dtype, y[0, 0, :2, :8])



# revision 2
# speedup vs baseline: 1.8061x; 1.8061x over previous
"""Int-infer matmul kernel v3 for trn2, 8 NeuronCores, data-parallel over (b,h).

reference: y = clip(round(matmul(clip(round(x1*r1)), clip(round(x2*r2))) / 16), -128, 127)
shapes: x1 [2,16,2048,64] f32, x2 [2,16,64,2048] f32 -> y [2,16,2048,2048] f32

v3 = v2's evict/ring structure with PE transposes (the xbar DMA-transpose is
~70x slower on real silicon than the cost model's 14ns/tile).

Per core (4 of the 32 (b,h) pairs, 2 supersteps of 2 pairs):
 - host prep: x1i = clip(round(x1*r1)) bf16, pre-swizzled to [128, 16*64]
   per pair (partition p = s%128, chunk c = s//128) so the load is a plain
   contiguous partition split; x2s = clip(round(x2*r2))/16 bf16 packed
   [128, 2048] per superstep (pair A rows 0:64, pair B 64:128). All values
   exact in bf16 (ints in [-128,127]; /16 = exponent shift). Folding /16
   into x2 makes the evict a pure f32->i8 convert.
 - prologue: PE-transpose all x1 chunks ([128,64] -> psum [64,128], pair A to
   partitions 0:64 / B to 64:128 via tile_position), copy psum->SBUF x1T
   (DVE 2x_1p bf16 / ACT, deficit-split). The tp pool (1 PSUM bank x2) closes
   before the main ring opens, so the ring still gets all 8 banks.
 - mains: K=64 row-packed matmuls via tile_position (0,0)/(64,0); PSUM is one
   [128, 4096] f32 ring: per m-tile, pair A fills [0:2048] (4x N=512), pair B
   fills [2048:4096]. Evicts = 4 staggered [128,1024] spans per m-round
   (PSUM f32 -> SBUF i8, RNE+saturate == clip(round(.))), deficit-balanced
   across ACT and DVE. 4 spans keep the PE refills off the evict critical
   path (fewer/bigger spans pipeline worse, measured in CoreSim).
 - output: i8 staging ring [128, 8192] (2 m-rounds), one DMA per pair per 2
   m-tiles (256KB, 2KB/descriptor); final round drains per-m to shrink the
   tail. Output upcast to f32 on host.
"""
import sys

sys.path.insert(0, "/opt/trn_rl_repo")

import numpy as np
import ml_dtypes
import concourse.bass as bass
import concourse.bacc as bacc
import concourse.mybir as mybir
import concourse.tile as tile
from concourse.bass_utils import run_bass_kernel_spmd
from concourse.masks import make_identity

F32 = mybir.dt.float32
BF16 = mybir.dt.bfloat16
I8 = mybir.dt.int8
AF = mybir.ActivationFunctionType

N_CORES = 8
PAIRS_PER_CORE = 4
S = 2048
D = 64
N_MM = 512    # moving free dim per matmul (one PSUM bank)
RING = 2 * S  # full PSUM: 4096 f32 per partition
NSPAN = 4
N_CHUNK = S // 128  # 16 s-chunks of 128 rows per pair


def build_program(repeat: int = 1, nspan=NSPAN) -> bass.Bass:
    nc = bacc.Bacc("TRN2", target_bir_lowering=False, debug=False, num_devices=N_CORES)
    n_ss = PAIRS_PER_CORE // 2
    # x1s[pair]: [128, 1024] bf16, swizzled: (p, c*64+d) = x1i[c*128+p, d]
    x1s = nc.dram_tensor("x1s", [PAIRS_PER_CORE, 128, N_CHUNK * D], BF16,
                         kind="ExternalInput").ap()
    x2p = nc.dram_tensor("x2p", [n_ss, 2 * D, S], BF16, kind="ExternalInput").ap()
    y = nc.dram_tensor("y", [PAIRS_PER_CORE, S, S], I8, kind="ExternalOutput").ap()

    if repeat > 1:
        nc.dram_tensor("rep_marker", [1, repeat], F32, kind="ExternalInput")

    if isinstance(nspan, (list, tuple)):
        bounds = [0]
        for sz in nspan:
            bounds.append(bounds[-1] + sz)
        assert bounds[-1] == RING
    else:
        bounds = [round(i * RING / nspan) for i in range(nspan + 1)]
    spans = list(zip(bounds[:-1], bounds[1:]))

    with tile.TileContext(nc) as tc:
      for _rep in range(repeat):
        ev = {"act": 0.0, "dve": 0.0}

        def assign(cost_act, cost_dve):
            if ev["act"] + cost_act <= ev["dve"] + cost_dve:
                ev["act"] += cost_act
                return "act"
            ev["dve"] += cost_dve
            return "dve"

        with (
            tc.tile_pool(name="x1raw", bufs=4) as x1raw_pool,
            tc.tile_pool(name="x1T", bufs=2) as x1T_pool,
            tc.tile_pool(name="x2t", bufs=2) as x2t_pool,
            tc.tile_pool(name="const", bufs=1) as const_pool,
        ):
            identity = const_pool.tile([128, 128], BF16)
            make_identity(nc, identity)

            x1Ts = []
            x2ts = []
            # prologue: load + PE-transpose all pairs' x1 into SBUF x1T tiles.
            # The dummy pool pins the tp tiles to PSUM banks 6-7 so the main
            # ring's early banks (cols 0:3072) carry no WAR against the
            # prologue - only mm0's last B-windows wait on the prologue tail.
            with tc.tile_pool(name="dummy", bufs=1, space="PSUM") as dummy_pool, \
                 tc.tile_pool(name="tpsum", bufs=2, space="PSUM") as tpsum_pool:
                dummy_pool.tile([128, 3072], F32, tag="dummy", name="dummy")
                raws = {}
                # x1 raws first (transposes gate everything); first pair's
                # raws chunked so the first transpose group starts early
                for p in range(PAIRS_PER_CORE):
                    raw = x1raw_pool.tile([128, N_CHUNK * D], BF16, tag="x1raw",
                                          name=f"x1raw{p}")
                    if p < 2:
                        h = N_CHUNK * D // 2
                        nc.sync.dma_start(out=raw[:, 0:h], in_=x1s[p, :, 0:h])
                        nc.sync.dma_start(out=raw[:, h:], in_=x1s[p, :, h:])
                    else:
                        nc.sync.dma_start(out=raw[:], in_=x1s[p])
                    raws[p] = raw
                for ss in range(n_ss):
                    x2t = x2t_pool.tile([128, S], BF16, tag="x2t", name=f"x2t{ss}")
                    nc.sync.dma_start(out=x2t[:, 0:S // 2], in_=x2p[ss, :, 0:S // 2])
                    nc.sync.dma_start(out=x2t[:, S // 2:S], in_=x2p[ss, :, S // 2:S])
                    x2ts.append(x2t)
                for ss in range(n_ss):
                    x1T = x1T_pool.tile([128, S], BF16, tag="x1T", name=f"x1T{ss}")
                    for g in range(N_CHUNK // 4):  # 4 tp tiles per superstep
                        tp = tpsum_pool.tile([128, 4 * 128], BF16, tag="tp",
                                             name=f"tp{ss}_{g}")
                        for j in range(4):
                            c = g * 4 + j
                            for half in (0, 1):
                                nc.tensor.transpose(
                                    tp[half * 64:half * 64 + 64, j * 128:(j + 1) * 128],
                                    raws[2 * ss + half][:, c * D:(c + 1) * D],
                                    identity[:],
                                    tile_position=(0, half * 64),
                                )
                        dst = x1T[:, g * 512:(g + 1) * 512]
                        # psum bf16 copy: DVE gets 2x_1p (2-byte packed)
                        if assign(1056.0, 694.0) == "act":
                            nc.scalar.activation(dst, tp[:], AF.Copy)
                        else:
                            nc.vector.tensor_copy(dst, tp[:])
                    x1Ts.append(x1T)

            with tc.tile_pool(name="ost", bufs=4) as ost_pool, \
                 tc.tile_pool(name="mpsum", bufs=1, space="PSUM") as mpsum_pool:
                ring = mpsum_pool.tile([128, RING], F32, tag="ring")

                for ss in range(n_ss):
                    x1T, x2t = x1Ts[ss], x2ts[ss]
                    for mq in range(8):  # 8 rounds of 2 m-tiles
                        last_round = ss == n_ss - 1 and mq == 7
                        ost = ost_pool.tile([128, 2 * RING], I8, tag="ost")
                        for mm in range(2):
                            m = 2 * mq + mm
                            mcols = slice(m * 128, (m + 1) * 128)
                            for half in (0, 1):
                                lo, hi = half * 64, half * 64 + 64
                                for w in range(S // N_MM):
                                    cols = slice(half * S + w * N_MM,
                                                 half * S + (w + 1) * N_MM)
                                    nc.tensor.matmul(
                                        ring[:, cols],
                                        lhsT=x1T[lo:hi, mcols],
                                        rhs=x2t[lo:hi, w * N_MM:(w + 1) * N_MM],
                                        start=True,
                                        stop=True,
                                        tile_position=(half * 64, 0),
                                    )
                            for (a, b) in spans:
                                dst = ost[:, mm * RING + a:mm * RING + b]
                                fd = b - a
                                if assign((172 + fd) / 1.2 + 59,
                                          (120 + fd) / 0.96 + 36) == "act":
                                    nc.scalar.activation(dst, ring[:, a:b], AF.Copy)
                                else:
                                    nc.vector.tensor_copy(dst, ring[:, a:b])
                            if last_round:
                                for half in (0, 1):
                                    p = 2 * ss + half
                                    # scalar (ACT) queue: free at the tail,
                                    # parallelizes the final DGE setups
                                    eng = nc.sync if half == 0 else nc.scalar
                                    eng.dma_start(
                                        out=y[p].rearrange("(m pp) c -> pp m c", pp=128)[
                                            :, 2 * mq + mm:2 * mq + mm + 1, :
                                        ],
                                        in_=ost.rearrange("p (m h c) -> p (m h) c",
                                                          m=2, h=2)[
                                            :, 2 * mm + half:2 * mm + half + 1, :
                                        ],
                                    )
                        if not last_round:
                            for half in (0, 1):
                                p = 2 * ss + half
                                nc.sync.dma_start(
                                    out=y[p].rearrange("(m pp) c -> pp m c", pp=128)[
                                        :, 2 * mq:2 * mq + 2, :
                                    ],
                                    in_=ost.rearrange("p (m h c) -> p (m h) c",
                                                      m=2, h=2)[:, half::2, :],
                                )

    nc.compile()
    return nc


_CACHE: dict = {}


def _prep(x1, x2, r1, r2):
    """Host-side quantizer rescale + layout packing (all values bf16-exact)."""
    x1i = np.clip(np.round(x1 * np.float32(r1)), -128.0, 127.0)
    x2s = np.clip(np.round(x2 * np.float32(r2)), -128.0, 127.0) * np.float32(1.0 / 16.0)
    n = x1.shape[0]
    # swizzle x1 per pair: [S, D] -> [(c p) d] -> [p, (c d)] with p = s%128
    x1v = x1i.reshape(n, N_CHUNK, 128, D).transpose(0, 2, 1, 3).reshape(n, 128, N_CHUNK * D)
    x2p = x2s.reshape(n // 2, 2 * D, S)
    return (
        np.ascontiguousarray(x1v).astype(ml_dtypes.bfloat16),
        np.ascontiguousarray(x2p).astype(ml_dtypes.bfloat16),
    )


def kernel(x1, x2, scale1_last_layer, scale_x1, scale2_last_layer, scale_x2):
    x1 = np.asarray(x1, dtype=np.float32)
    x2 = np.asarray(x2, dtype=np.float32)
    r1 = float(np.float32(scale1_last_layer) / np.float32(scale_x1))
    r2 = float(np.float32(scale2_last_layer) / np.float32(scale_x2))

    if "nc" not in _CACHE:
        _CACHE["nc"] = build_program()
    nc = _CACHE["nc"]

    b, h = x1.shape[0], x1.shape[1]
    x1r = x1.reshape(b * h, S, D)
    x2r = x2.reshape(b * h, D, S)
    in_maps = []
    for c in range(N_CORES):
        sl = slice(c * PAIRS_PER_CORE, (c + 1) * PAIRS_PER_CORE)
        x1s_, x2p_ = _prep(x1r[sl], x2r[sl], r1, r2)
        in_maps.append({"x1s": x1s_, "x2p": x2p_})
    res = run_bass_kernel_spmd(nc, in_maps, list(range(N_CORES)))
    out = np.concatenate([r["y"] for r in res.results], axis=0)
    return out.reshape(b, h, S, S).astype(np.float32)


if __name__ == "__main__":
    rng = np.random.default_rng(0)
    x1 = np.round(np.clip(rng.normal(size=(2, 16, S, D)) * 40.0, -128, 127)).astype(np.float32)
    x2 = np.round(np.clip(rng.normal(size=(2, 16, D, S)) * 40.0, -128, 127)).astype(np.float32)
    y = kernel(x1, x2, np.float32(0.1), np.float32(0.05), np.float32(0.08), np.float32(0.04))
    print("out", y.shape, y.dtype, y[0, 0, :2, :8])


# revision 14
# speedup vs baseline: 2.6222x; 1.4519x over previous
"""Int-infer matmul kernel v3 for trn2, 8 NeuronCores, data-parallel over (b,h).

reference: y = clip(round(matmul(clip(round(x1*r1)), clip(round(x2*r2))) / 16), -128, 127)
shapes: x1 [2,16,2048,64] f32, x2 [2,16,64,2048] f32 -> y [2,16,2048,2048] f32

v3 = v2's evict/ring structure with PE transposes (the xbar DMA-transpose is
~70x slower on real silicon than the cost model's 14ns/tile).

Per core (4 of the 32 (b,h) pairs, 2 supersteps of 2 pairs):
 - host prep: x1i = clip(round(x1*r1)) bf16, pre-swizzled to [128, 16*64]
   per pair (partition p = s%128, chunk c = s//128) so the load is a plain
   contiguous partition split; x2s = clip(round(x2*r2))/16 bf16 packed
   [128, 2048] per superstep (pair A rows 0:64, pair B 64:128). All values
   exact in bf16 (ints in [-128,127]; /16 = exponent shift). Folding /16
   into x2 makes the evict a pure f32->i8 convert.
 - prologue: PE-transpose all x1 chunks ([128,64] -> psum [64,128], pair A to
   partitions 0:64 / B to 64:128 via tile_position), copy psum->SBUF x1T
   (DVE 2x_1p bf16 / ACT, deficit-split). The tp pool (1 PSUM bank x2) closes
   before the main ring opens, so the ring still gets all 8 banks.
 - mains: K=64 row-packed matmuls via tile_position (0,0)/(64,0); PSUM is one
   [128, 4096] f32 ring: per m-tile, pair A fills [0:2048] (4x N=512), pair B
   fills [2048:4096]. Evicts = 4 staggered [128,1024] spans per m-round
   (PSUM f32 -> SBUF i8, RNE+saturate == clip(round(.))), deficit-balanced
   across ACT and DVE. 4 spans keep the PE refills off the evict critical
   path (fewer/bigger spans pipeline worse, measured in CoreSim).
 - output: i8 staging ring [128, 8192] (2 m-rounds), one DMA per pair per 2
   m-tiles (256KB, 2KB/descriptor); final round drains per-m to shrink the
   tail. Output upcast to f32 on host.
"""
import sys

sys.path.insert(0, "/opt/trn_rl_repo")

import numpy as np
import ml_dtypes
import concourse.bass as bass
import concourse.bacc as bacc
import concourse.mybir as mybir
import concourse.tile as tile
from concourse.bass_utils import run_bass_kernel_spmd
from concourse.masks import make_identity

F32 = mybir.dt.float32
BF16 = mybir.dt.bfloat16
I8 = mybir.dt.int8
AF = mybir.ActivationFunctionType

N_CORES = 8
PAIRS_PER_CORE = 4
S = 2048
D = 64
N_MM = 512    # moving free dim per matmul (one PSUM bank)
RING = 2 * S  # full PSUM: 4096 f32 per partition
NSPAN = 4
N_CHUNK = S // 128  # 16 s-chunks of 128 rows per pair


def build_program(repeat: int = 1, nspan=NSPAN) -> bass.Bass:
    nc = bacc.Bacc("TRN2", target_bir_lowering=False, debug=False, num_devices=N_CORES)
    n_ss = PAIRS_PER_CORE // 2
    # x1s[pair]: [128, 1024] bf16, swizzled: (p, c*64+d) = x1i[c*128+p, d]
    x1s = nc.dram_tensor("x1s", [PAIRS_PER_CORE, 128, N_CHUNK * D], BF16,
                         kind="ExternalInput").ap()
    x2p = nc.dram_tensor("x2p", [n_ss, 2 * D, S], BF16, kind="ExternalInput").ap()
    y = nc.dram_tensor("y", [PAIRS_PER_CORE, S, S], I8, kind="ExternalOutput").ap()

    if repeat > 1:
        nc.dram_tensor("rep_marker", [1, repeat], F32, kind="ExternalInput")

    if isinstance(nspan, (list, tuple)):
        bounds = [0]
        for sz in nspan:
            bounds.append(bounds[-1] + sz)
        assert bounds[-1] == RING
    else:
        bounds = [round(i * RING / nspan) for i in range(nspan + 1)]
    spans = list(zip(bounds[:-1], bounds[1:]))

    with tile.TileContext(nc) as tc:
      for _rep in range(repeat):
        ev = {"act": 0.0, "dve": 0.0}

        def assign(cost_act, cost_dve):
            if ev["act"] + cost_act <= ev["dve"] + cost_dve:
                ev["act"] += cost_act
                return "act"
            ev["dve"] += cost_dve
            return "dve"

        with (
            tc.tile_pool(name="x1raw", bufs=4) as x1raw_pool,
            tc.tile_pool(name="x1T", bufs=2) as x1T_pool,
            tc.tile_pool(name="x2t", bufs=2) as x2t_pool,
            tc.tile_pool(name="const", bufs=1) as const_pool,
        ):
            identity = const_pool.tile([128, 128], BF16)
            make_identity(nc, identity)

            x1Ts = []
            x2ts = []
            # prologue: load + PE-transpose all pairs' x1 into SBUF x1T tiles.
            # The dummy pool pins the tp tiles to PSUM banks 6-7 so the main
            # ring's early banks (cols 0:3072) carry no WAR against the
            # prologue - only mm0's last B-windows wait on the prologue tail.
            with tc.tile_pool(name="dummy", bufs=1, space="PSUM") as dummy_pool, \
                 tc.tile_pool(name="tpsum", bufs=2, space="PSUM") as tpsum_pool:
                dummy_pool.tile([128, 3072], F32, tag="dummy", name="dummy")
                raws = {}
                # x1 raws first (transposes gate everything); first pair's
                # raws chunked so the first transpose group starts early
                for p in range(PAIRS_PER_CORE):
                    raws[p] = x1raw_pool.tile([128, N_CHUNK * D], BF16, tag="x1raw",
                                              name=f"x1raw{p}")
                # ss0 pair chunks interleaved: the first transpose groups
                # need the leading cols of BOTH pairs, so land those together
                q = N_CHUNK * D // 2
                for ci in range(2):
                    for p in (0, 1):
                        nc.sync.dma_start(out=raws[p][:, ci * q:(ci + 1) * q],
                                          in_=x1s[p, :, ci * q:(ci + 1) * q])
                for p in (2, 3):
                    nc.sync.dma_start(out=raws[p][:], in_=x1s[p])
                for ss in range(n_ss):
                    x2t = x2t_pool.tile([128, S], BF16, tag="x2t", name=f"x2t{ss}")
                    nc.sync.dma_start(out=x2t[:, 0:S // 2], in_=x2p[ss, :, 0:S // 2])
                    nc.sync.dma_start(out=x2t[:, S // 2:S], in_=x2p[ss, :, S // 2:S])
                    x2ts.append(x2t)
                for ss in range(n_ss):
                    x1T = x1T_pool.tile([128, S], BF16, tag="x1T", name=f"x1T{ss}")
                    for g in range(N_CHUNK // 4):  # 4 tp tiles per superstep
                        tp = tpsum_pool.tile([128, 4 * 128], BF16, tag="tp",
                                             name=f"tp{ss}_{g}")
                        for j in range(4):
                            c = g * 4 + j
                            for half in (0, 1):
                                nc.tensor.transpose(
                                    tp[half * 64:half * 64 + 64, j * 128:(j + 1) * 128],
                                    raws[2 * ss + half][:, c * D:(c + 1) * D],
                                    identity[:],
                                    tile_position=(0, half * 64),
                                )
                        dst = x1T[:, g * 512:(g + 1) * 512]
                        # psum bf16 copy: DVE gets 2x_1p (2-byte packed)
                        if assign(1056.0, 694.0) == "act":
                            nc.scalar.activation(dst, tp[:], AF.Copy)
                        else:
                            nc.vector.tensor_copy(dst, tp[:])
                    x1Ts.append(x1T)

            with tc.tile_pool(name="ost", bufs=4) as ost_pool, \
                 tc.tile_pool(name="mpsum", bufs=1, space="PSUM") as mpsum_pool:
                ring = mpsum_pool.tile([128, RING], F32, tag="ring")

                for ss in range(n_ss):
                    x1T, x2t = x1Ts[ss], x2ts[ss]
                    for mq in range(8):  # 8 rounds of 2 m-tiles
                        last_round = ss == n_ss - 1 and mq == 7
                        ost = ost_pool.tile([128, 2 * RING], I8, tag="ost")
                        for mm in range(2):
                            m = 2 * mq + mm
                            mcols = slice(m * 128, (m + 1) * 128)
                            for half in (0, 1):
                                lo, hi = half * 64, half * 64 + 64
                                for w in range(S // N_MM):
                                    cols = slice(half * S + w * N_MM,
                                                 half * S + (w + 1) * N_MM)
                                    nc.tensor.matmul(
                                        ring[:, cols],
                                        lhsT=x1T[lo:hi, mcols],
                                        rhs=x2t[lo:hi, w * N_MM:(w + 1) * N_MM],
                                        start=True,
                                        stop=True,
                                        tile_position=(half * 64, 0),
                                    )
                            cur_spans = spans
                            if ss == 0 and mq == 0 and mm == 0:
                                cur_spans = [(i * 512, (i + 1) * 512) for i in range(8)]
                            for (a, b) in cur_spans:
                                dst = ost[:, mm * RING + a:mm * RING + b]
                                fd = b - a
                                if assign(1.02 * ((172 + fd) / 1.2 + 59),
                                          (120 + fd) / 0.96 + 36) == "act":
                                    nc.scalar.activation(dst, ring[:, a:b], AF.Copy)
                                else:
                                    nc.vector.tensor_copy(dst, ring[:, a:b])
                            if last_round:
                                for half in (0, 1):
                                    p = 2 * ss + half
                                    nc.sync.dma_start(
                                        out=y[p].rearrange("(m pp) c -> pp m c", pp=128)[
                                            :, 2 * mq + mm:2 * mq + mm + 1, :
                                        ],
                                        in_=ost.rearrange("p (m h c) -> p (m h) c",
                                                          m=2, h=2)[
                                            :, 2 * mm + half:2 * mm + half + 1, :
                                        ],
                                    )
                        if not last_round:
                            for half in (0, 1):
                                p = 2 * ss + half
                                nc.sync.dma_start(
                                    out=y[p].rearrange("(m pp) c -> pp m c", pp=128)[
                                        :, 2 * mq:2 * mq + 2, :
                                    ],
                                    in_=ost.rearrange("p (m h c) -> p (m h) c",
                                                      m=2, h=2)[:, half::2, :],
                                )

    nc.compile()
    return nc


_CACHE: dict = {}


def _prep(x1, x2, r1, r2):
    """Host-side quantizer rescale + layout packing (all values bf16-exact)."""
    x1i = np.clip(np.round(x1 * np.float32(r1)), -128.0, 127.0)
    x2s = np.clip(np.round(x2 * np.float32(r2)), -128.0, 127.0) * np.float32(1.0 / 16.0)
    n = x1.shape[0]
    # swizzle x1 per pair: [S, D] -> [(c p) d] -> [p, (c d)] with p = s%128
    x1v = x1i.reshape(n, N_CHUNK, 128, D).transpose(0, 2, 1, 3).reshape(n, 128, N_CHUNK * D)
    x2p = x2s.reshape(n // 2, 2 * D, S)
    return (
        np.ascontiguousarray(x1v).astype(ml_dtypes.bfloat16),
        np.ascontiguousarray(x2p).astype(ml_dtypes.bfloat16),
    )


def kernel(x1, x2, scale1_last_layer, scale_x1, scale2_last_layer, scale_x2):
    x1 = np.asarray(x1, dtype=np.float32)
    x2 = np.asarray(x2, dtype=np.float32)
    r1 = float(np.float32(scale1_last_layer) / np.float32(scale_x1))
    r2 = float(np.float32(scale2_last_layer) / np.float32(scale_x2))

    if "nc" not in _CACHE:
        _CACHE["nc"] = build_program()
    nc = _CACHE["nc"]

    b, h = x1.shape[0], x1.shape[1]
    x1r = x1.reshape(b * h, S, D)
    x2r = x2.reshape(b * h, D, S)
    in_maps = []
    for c in range(N_CORES):
        sl = slice(c * PAIRS_PER_CORE, (c + 1) * PAIRS_PER_CORE)
        x1s_, x2p_ = _prep(x1r[sl], x2r[sl], r1, r2)
        in_maps.append({"x1s": x1s_, "x2p": x2p_})
    res = run_bass_kernel_spmd(nc, in_maps, list(range(N_CORES)))
    out = np.concatenate([r["y"] for r in res.results], axis=0)
    return out.reshape(b, h, S, S).astype(np.float32)


if __name__ == "__main__":
    rng = np.random.default_rng(0)
    x1 = np.round(np.clip(rng.normal(size=(2, 16, S, D)) * 40.0, -128, 127)).astype(np.float32)
    x2 = np.round(np.clip(rng.normal(size=(2, 16, D, S)) * 40.0, -128, 127)).astype(np.float32)
    y = kernel(x1, x2, np.float32(0.1), np.float32(0.05), np.float32(0.08), np.float32(0.04))
    print("out", y.shape, y.dtype, y[0, 0, :2, :8])
